# revision 42
# baseline (speedup 1.0000x reference)
"""Multi-head attention block (nn_Attention) on 8 Trainium2 NeuronCores.

Reference computation (fp32):
    qkv = x @ w_qkv;  q,k,v = split(qkv);  per-head softmax(q k^T / sqrt(d)) v
    out = concat_heads @ w_out + b_out
Shapes: x [4, 2048, 1024], w_qkv [1024, 3072], w_out [1024, 1024], b_out [1024].

Sharding: DP over batch (4) x TP over head-groups (2) = 8 cores.
Core c handles batch c//2 and heads [8*(c%2), 8*(c%2)+8). Each core computes a
partial output projection over its 8 heads; the host sums the two partials per
batch and adds b_out (the unshard/gather step). No on-device collectives.

Per-core kernel (all-fp16 data path, fp32 PSUM accumulation):
  head:  x arrives fp16 (host-cast); x^T built by 4 DMA-xbar transposes (one
         per 512-token chunk, contiguous DRAM source, 3D SBUF dest) -- no PE
         transposes, no PSUM evacuation copies. V = x w_v (+ones col ->
         V_aug); Q^T/K^T chunks for head-pair 0 overlap the transposes.
  attn:  per head-pair: S^T = K^T.T Q^T (row-tiled K=64 matmul pairs run
         concurrently on the PE); P^T = exp(S^T/8) split across TWO engines:
         ScalarE LUT exp for 10/16 key blocks, DVE for 6/16 via a single
         tensor_scalar (i16 = S*A + B, bitcast fp16 == 2^(S*log2e/8),
         Schraudolph; +-3% sawtooth that washes out in the softmax ratio);
         O_aug^T = V_aug^T P^T accumulated in PSUM, row 64 = denominators.
         Normalization (SBUF-stage + reciprocal_approx_fast + GpSimd
         partition broadcast + DVE multiply) is LAGGED: its ops are emitted
         interleaved into the next i-block's loop so their latency never
         blocks the exp/matmul pipeline (engines execute their queues in
         order). Next pair's QKV matmuls are woven into spare PE cycles.
  tail:  folded into the last head-pair's loop the same way: after each
         i-block's normalization, its out-projection matmuls, PSUM
         evacuation, and output DMA are queued behind it.
"""
import sys

sys.path.insert(0, "/opt/trn_rl_repo")

import numpy as np

import concourse.bacc as bacc
import concourse.mybir as mybir
from concourse.tile import TileContext
from concourse.bass_utils import run_bass_kernel_spmd

F32 = mybir.dt.float32
F16 = mybir.dt.float16
I16 = mybir.dt.int16
EXP = mybir.ActivationFunctionType.Exp
MULT = mybir.AluOpType.mult
ADD = mybir.AluOpType.add

T = 2048      # tokens per core (one batch element)
E = 1024      # model dim
HPC = 8       # heads per core
D = 64        # head dim
SCALE = D ** -0.5
NEC = E // 128   # 8 e-chunks
NI = 4           # i blocks of 512 (attention query cols)
NJ = 16          # j blocks of 128 (attention key rows)

# DVE-exp (Schraudolph) split: which j blocks go to the DVE instead of ScalarE.
# Spread so ScalarE never runs more than 2-3 consecutive tiles (a longer burst
# stalls the S matmuls through the 2-deep sAB PSUM rotation).
DVE_JB = frozenset((2, 4, 6, 8, 10, 12))
EXP_A = float(np.log2(np.e) / 8.0 * 1024.0)   # 184.66496...
EXP_B = 15360.0 - 44.5                        # fp16 bias 15<<10, sawtooth-centering

_CACHED_NC = None


def build_nc():
    nc = bacc.Bacc("TRN2", target_bir_lowering=False, debug=False, num_devices=8)
    x_d = nc.declare_dram_parameter("x", [T, E], F16, isOutput=False)
    wqk_d = nc.declare_dram_parameter("wqk", [E, 1024], F16, isOutput=False)
    wv_d = nc.declare_dram_parameter("wv", [E, 512], F16, isOutput=False)
    wo_d = nc.declare_dram_parameter("wo", [512, E], F16, isOutput=False)
    out_d = nc.declare_dram_parameter("out", [T, E], F32, isOutput=True)

    with TileContext(nc) as tc:
        with (
            tc.tile_pool(name="xph", bufs=1) as x_pool,
            tc.tile_pool(name="vaugp", bufs=1) as vaug_pool,
            tc.tile_pool(name="wvp", bufs=1) as wv_pool,
            tc.tile_pool(name="wop", bufs=1) as wo_pool,
            tc.tile_pool(name="otp", bufs=1) as ot_pool,
            tc.tile_pool(name="qkt", bufs=2) as qkt_pool,
            tc.tile_pool(name="wstr", bufs=4) as w_pool,
            tc.tile_pool(name="pt", bufs=6) as pt_pool,
            tc.tile_pool(name="rcp", bufs=2) as rc_pool,
            tc.tile_pool(name="rbp", bufs=2) as rb_pool,
            tc.tile_pool(name="ost2", bufs=3) as out2_pool,
            tc.tile_pool(name="qk_ps", bufs=2, space="PSUM") as qk_psum,
            tc.tile_pool(name="s_ps", bufs=2, space="PSUM") as s_psum,
            tc.tile_pool(name="oa_ps", bufs=2, space="PSUM") as oa_psum,
        ):
            # x^T as 4 per-token-chunk tiles: xTt[tcb][p, ec, t] = x[tcb*512+t,
            # ec*128+p]. Separate tiles per chunk avoid write-after-read
            # serialization between the transpose DMAs and the V/QK matmuls.
            xTt = [
                x_pool.tile([128, NEC * 512], F16, tag=f"xT{tcb}", name=f"xT{tcb}")
                for tcb in range(4)
            ]
            xTv = [t[:].rearrange("p (e t) -> p e t", t=512) for t in xTt]
            # V_aug padded to 128 cols/head: PV weight slices become 16B-
            # aligned and FWL-eligible (128 cols), so their LDWEIGHTS hide
            # behind the streaming matmuls. Cols 65..127 are zero; the extra
            # PSUM output rows 65..127 are never read.
            vaug = [
                vaug_pool.tile([128, HPC * 128], F16, tag=f"va{jb}", name=f"va{jb}")
                for jb in range(NJ)
            ]
            wv_sb = wv_pool.tile([128, NEC * 512], F16, tag="wv")
            wo_sb = [
                wo_pool.tile([128, E], F16, tag=f"wo{hc}", name=f"wo{hc}")
                for hc in range(4)
            ]
            oT = [
                ot_pool.tile([128, T], F16, tag=f"oT{hc}", name=f"oT{hc}")
                for hc in range(4)
            ]

            # ---------------- weight DMAs (gpsimd queue) --------------------
            for ec in range(NEC):
                nc.gpsimd.dma_start(
                    out=wv_sb[:, ec * 512 : (ec + 1) * 512],
                    in_=wv_d[ec * 128 : (ec + 1) * 128, :],
                )
            for hc in range(4):
                nc.gpsimd.dma_start(
                    out=wo_sb[hc][:], in_=wo_d[hc * 128 : (hc + 1) * 128, :]
                )

            # ---------------- x^T: one xbar transpose per token chunk -------
            for tcb in range(4):
                nc.sync.dma_start_transpose(
                    xTv[tcb], x_d[tcb * 512 : (tcb + 1) * 512, :]
                )

            def qk_pair_closures(pair, qp, kp):
                """Closures computing Q^T/K^T chunks for head pair `pair` into
                qp/kp (fp16). Entries: [load_w, q-mm0..3, q-evac0..3,
                k-mm0..3, k-evac0..3] -- matmul groups and their PSUM
                evacuations are separate so the evac's latency can lag."""
                wq = w_pool.tile([128, E], F16, tag="wcb", name=f"wq{pair}")
                wk = w_pool.tile([128, E], F16, tag="wcb", name=f"wk{pair}")
                cls = []

                def load_w():
                    for half, wt in ((0, wq), (1, wk)):
                        cols = slice(
                            512 * half + pair * 128, 512 * half + (pair + 1) * 128
                        )
                        nc.gpsimd.dma_start(
                            out=wt[:].rearrange("p (e c) -> p e c", c=128),
                            in_=wqk_d[:, cols].rearrange("(e p) c -> p e c", p=128),
                        )

                cls.append(load_w)
                evac = [0]
                for wcb, dst in ((wq, qp), (wk, kp)):
                    for ib in range(NI):
                        ps_ref = []

                        def grp_a(wcb=wcb, ib=ib, ps_ref=ps_ref):
                            ps = qk_psum.tile([128, 512], F32, tag="qkp")
                            ps_ref.append(ps)
                            for ec in range(4):
                                nc.tensor.matmul(
                                    ps[:],
                                    wcb[:, ec * 128 : (ec + 1) * 128],
                                    xTv[ib][:, ec, :],
                                    start=(ec == 0),
                                    stop=False,
                                )

                        def grp_b(wcb=wcb, dst=dst, ib=ib, ps_ref=ps_ref):
                            ps = ps_ref[0]
                            for ec in range(4, NEC):
                                nc.tensor.matmul(
                                    ps[:],
                                    wcb[:, ec * 128 : (ec + 1) * 128],
                                    xTv[ib][:, ec, :],
                                    start=False,
                                    stop=(ec == NEC - 1),
                                )
                            if evac[0] % 2 == 0:
                                nc.vector.tensor_copy(
                                    dst[:, ib * 512 : (ib + 1) * 512], ps[:]
                                )
                            else:
                                nc.scalar.copy(
                                    dst[:, ib * 512 : (ib + 1) * 512], ps[:]
                                )
                            evac[0] += 1

                        cls.append(grp_a)
                        cls.append(grp_b)
                return cls

            # Keep-warm filler: the PE would otherwise idle ~15us waiting for
            # the first x transpose, HAM-throttle to 1.2 GHz, and run the
            # whole head at half clock. These matmuls (garbage results into
            # the attention-phase-only oa pool, never read) execute during
            # the DMA wait and keep the clock gate at 8/8.
            for _ in range(48):
                dps = oa_psum.tile([128, 512], F32, tag="oa", name="warm")
                nc.tensor.matmul(
                    dps[:], wv_sb[:, 0:128], wv_sb[:, 0:512],
                    start=True, stop=True,
                )

            # ---------------- head: V_aug and qk pair 0 ---------------------
            def emit_v(jb):
                vview = vaug[jb][:].rearrange("p (h c) -> p h c", c=128)
                nc.vector.memset(vaug[jb][:], 0.0)
                nc.vector.memset(vview[:, :, 64:65], 1.0)
                ps = qk_psum.tile([128, 512], F32, tag="qkp")
                for ec in range(NEC):
                    nc.tensor.matmul(
                        ps[:],
                        xTv[jb // 4][:, ec, (jb % 4) * 128 : (jb % 4 + 1) * 128],
                        wv_sb[:, ec * 512 : (ec + 1) * 512],
                        start=(ec == 0),
                        stop=(ec == NEC - 1),
                    )
                nc.vector.tensor_copy(
                    vview[:, :, 0:64], ps[:].rearrange("p (h c) -> p h c", c=64)
                )

            for jb in range(NJ):
                emit_v(jb)
                if jb % 4 == 3 and jb < NJ - 1:
                    # bridge the gap to the next transpose's arrival so the
                    # HAM activity window never goes fully idle
                    for _ in range(14):
                        dps = oa_psum.tile([128, 512], F32, tag="oa", name="warm")
                        nc.tensor.matmul(
                            dps[:], wv_sb[:, 0:128], wv_sb[:, 0:512],
                            start=True, stop=True,
                        )

            qp0 = qkt_pool.tile([128, T], F16, tag="qp", name="qp0")
            kp0 = qkt_pool.tile([128, T], F16, tag="kp", name="kp0")
            cls0 = qk_pair_closures(0, qp0, kp0)
            cls0[0]()          # load_w
            cls0[1](); cls0[2]()   # q-ib0 (both halves)
            for fn in cls0[9:17]:
                fn()           # k-ib0..3
            qk0_deferred = cls0[3:9]

            # ---------------- attention + lagged norm / QKV / out-proj ------
            prj_cnt = [0]

            def prj_closures(ib):
                """out-projection closures for i-block ib (hc3 only): per
                token block, [matmul-group, evac+dma] as separate entries."""
                cls = []
                for tb in range(4 * ib, 4 * ib + 4):
                    trows = slice(tb * 128, (tb + 1) * 128)
                    for eb in range(2):
                        ecols = slice(eb * 512, (eb + 1) * 512)
                        ps_ref = []

                        def mms(trows=trows, ecols=ecols, ps_ref=ps_ref):
                            ps = qk_psum.tile([128, 512], F32, tag="qkp")
                            ps_ref.append(ps)
                            for hcc in range(4):
                                nc.tensor.matmul(
                                    ps[:],
                                    oT[hcc][:, trows],
                                    wo_sb[hcc][:, ecols],
                                    start=(hcc == 0),
                                    stop=(hcc == 3),
                                )

                        def ev(trows=trows, ecols=ecols, ps_ref=ps_ref):
                            ps = ps_ref[0]
                            ot = out2_pool.tile([128, 512], F32, tag="ost")
                            if prj_cnt[0] % 2 == 0:
                                nc.scalar.copy(ot[:], ps[:])
                            else:
                                nc.vector.tensor_copy(ot[:], ps[:])
                            deng = nc.sync if prj_cnt[0] % 2 == 0 else nc.gpsimd
                            deng.dma_start(out=out_d[trows, ecols], in_=ot[:])
                            prj_cnt[0] += 1

                        cls.append(mms)
                        cls.append(ev)
                return cls

            def norm_closures(hc, ib, oaugA, oaugB):
                """Normalization for (hc, ib): per head [stage+recip,
                broadcast, multiply] as 3 lag-queue entries."""
                icols = slice(ib * 512, (ib + 1) * 512)
                cls = []
                for oaug, rowoff in ((oaugA, 0), (oaugB, 64)):
                    st = {}

                    def recip(oaug=oaug, st=st):
                        # reciprocal_approx_fast misreads PSUM; stage via SBUF
                        dr = rc_pool.tile([1, 512], F32, tag="dr")
                        nc.vector.tensor_copy(dr[0:1, :], oaug[64:65, :])
                        rc0 = rc_pool.tile([1, 512], F32, tag="rc0")
                        nc.vector.reciprocal_approx_fast(rc0[0:1, :], dr[0:1, :])
                        st["rc0"] = rc0

                    def bcast(st=st):
                        rbs = rb_pool.tile([64, 512], F32, tag="rbs")
                        nc.gpsimd.partition_broadcast(rbs[:], st["rc0"][0:1, :])
                        st["rbs"] = rbs

                    def mul(oaug=oaug, rowoff=rowoff, st=st):
                        nc.vector.tensor_mul(
                            oT[hc][rowoff : rowoff + 64, icols],
                            oaug[0:64, :],
                            st["rbs"][:],
                        )

                    cls.extend((recip, bcast, mul))
                return cls

            qp, kp = qp0, kp0
            pending = []
            lagq = []
            for hc in range(4):
                hA, hB = 2 * hc, 2 * hc + 1
                if hc < 3:
                    qn = qkt_pool.tile([128, T], F16, tag="qp", name=f"qp{hc+1}")
                    kn = qkt_pool.tile([128, T], F16, tag="kp", name=f"kp{hc+1}")
                    pending = qk_pair_closures(hc + 1, qn, kn)
                    if hc == 0:
                        pending = qk0_deferred + pending
                else:
                    qn = kn = None
                steps = 0
                for ib in range(NI):
                    icols = slice(ib * 512, (ib + 1) * 512)
                    oaugA = oa_psum.tile([128, 512], F32, tag="oa", name="oaugA")
                    oaugB = oa_psum.tile([128, 512], F32, tag="oa", name="oaugB")
                    prev_pAB = None

                    def emit_pv(pAB, jb):
                        nc.tensor.matmul(
                            oaugA[:],
                            vaug[jb][:, hA * 128 : hA * 128 + 128],
                            pAB[:, 0:512],
                            start=(jb == 0), stop=(jb == NJ - 1),
                        )
                        nc.tensor.matmul(
                            oaugB[:],
                            vaug[jb][:, hB * 128 : hB * 128 + 128],
                            pAB[:, 512:1024],
                            start=(jb == 0), stop=(jb == NJ - 1),
                        )

                    for jb in range(NJ):
                        jcols = slice(jb * 128, (jb + 1) * 128)
                        sAB = s_psum.tile([128, 1024], F32, tag="sAB")
                        nc.tensor.matmul(
                            sAB[:, 0:512], kp[0:64, jcols], qp[0:64, icols],
                            start=True, stop=True,
                        )
                        nc.tensor.matmul(
                            sAB[:, 512:1024], kp[64:128, jcols],
                            qp[64:128, icols],
                            start=True, stop=True,
                        )
                        pAB = pt_pool.tile([128, 1024], F16, tag="pAB")
                        if jb in DVE_JB:
                            nc.vector.tensor_scalar(
                                pAB[:].bitcast(I16), sAB[:], EXP_A, EXP_B,
                                MULT, ADD,
                            )
                        else:
                            nc.scalar.activation(pAB[:], sAB[:], EXP, scale=SCALE)
                        if prev_pAB is not None:
                            emit_pv(prev_pAB, jb - 1)
                        prev_pAB = pAB
                        # lagged work: norm/out-proj first (short queue),
                        # then the next pair's QKV weave
                        steps += 1
                        if lagq:
                            lagq.pop(0)()
                        if len(lagq) > 6:
                            lagq.pop(0)()
                        if pending and steps % 3 == 0:
                            pending.pop(0)()
                        # drain oversized backlogs (hc0 carries pair0's
                        # deferred work too) before the hc boundary, where a
                        # flush would stall the PE past the HAM window
                        if len(pending) > 17 and steps % 3 == 1:
                            pending.pop(0)()
                    emit_pv(prev_pAB, NJ - 1)

                    lagq.extend(norm_closures(hc, ib, oaugA, oaugB))
                    if hc == 3:
                        lagq.extend(prj_closures(ib))
                for fn in pending:
                    fn()
                pending = []
                qp, kp = qn, kn
            for fn in lagq:
                fn()

    nc.compile()
    return nc


def get_nc():
    global _CACHED_NC
    if _CACHED_NC is None:
        _CACHED_NC = build_nc()
    return _CACHED_NC


def make_in_maps(x, w_qkv, w_out):
    in_maps = []
    for c in range(8):
        bi, hg = divmod(c, 2)
        wqk_c = np.concatenate(
            [
                w_qkv[:, hg * 512 : hg * 512 + 512],
                w_qkv[:, 1024 + hg * 512 : 1024 + hg * 512 + 512],
            ],
            axis=1,
        )
        in_maps.append(
            {
                "x": np.ascontiguousarray(x[bi]).astype(np.float16),
                "wqk": np.ascontiguousarray(wqk_c).astype(np.float16),
                "wv": np.ascontiguousarray(
                    w_qkv[:, 2048 + hg * 512 : 2048 + hg * 512 + 512]
                ).astype(np.float16),
                "wo": np.ascontiguousarray(
                    w_out[hg * 512 : hg * 512 + 512, :]
                ).astype(np.float16),
            }
        )
    return in_maps


def kernel(x, w_qkv, w_out, b_out):
    x = np.asarray(x, dtype=np.float32)
    w_qkv = np.asarray(w_qkv, dtype=np.float32)
    w_out = np.asarray(w_out, dtype=np.float32)
    b_out = np.asarray(b_out, dtype=np.float32)
    nc = get_nc()
    res = run_bass_kernel_spmd(nc, make_in_maps(x, w_qkv, w_out), list(range(8)))
    parts = [res.results[c]["out"] for c in range(8)]
    out = np.stack([parts[2 * bi] + parts[2 * bi + 1] for bi in range(4)])
    out += b_out[None, None, :]
    return out.astype(np.float32)


# revision 43
# speedup vs baseline: 1.0062x; 1.0062x over previous
"""Multi-head attention block (nn_Attention) on 8 Trainium2 NeuronCores.

Reference computation (fp32):
    qkv = x @ w_qkv;  q,k,v = split(qkv);  per-head softmax(q k^T / sqrt(d)) v
    out = concat_heads @ w_out + b_out
Shapes: x [4, 2048, 1024], w_qkv [1024, 3072], w_out [1024, 1024], b_out [1024].

Sharding: DP over batch (4) x TP over head-groups (2) = 8 cores.
Core c handles batch c//2 and heads [8*(c%2), 8*(c%2)+8). Each core computes a
partial output projection over its 8 heads; the host sums the two partials per
batch and adds b_out (the unshard/gather step). No on-device collectives.

Per-core kernel (all-fp16 data path, fp32 PSUM accumulation):
  head:  x arrives fp16 (host-cast); x^T built by 4 DMA-xbar transposes (one
         per 512-token chunk, contiguous DRAM source, 3D SBUF dest) -- no PE
         transposes, no PSUM evacuation copies. V = x w_v (+ones col ->
         V_aug); Q^T/K^T chunks for head-pair 0 overlap the transposes.
  attn:  per head-pair: S^T = K^T.T Q^T (row-tiled K=64 matmul pairs run
         concurrently on the PE); P^T = exp(S^T/8) split across TWO engines:
         ScalarE LUT exp for 10/16 key blocks, DVE for 6/16 via a single
         tensor_scalar (i16 = S*A + B, bitcast fp16 == 2^(S*log2e/8),
         Schraudolph; +-3% sawtooth that washes out in the softmax ratio);
         O_aug^T = V_aug^T P^T accumulated in PSUM, row 64 = denominators.
         Normalization (SBUF-stage + reciprocal_approx_fast + GpSimd
         partition broadcast + DVE multiply) is LAGGED: its ops are emitted
         interleaved into the next i-block's loop so their latency never
         blocks the exp/matmul pipeline (engines execute their queues in
         order). Next pair's QKV matmuls are woven into spare PE cycles.
  tail:  folded into the last head-pair's loop the same way: after each
         i-block's normalization, its out-projection matmuls, PSUM
         evacuation, and output DMA are queued behind it.
"""
import sys

sys.path.insert(0, "/opt/trn_rl_repo")

import numpy as np

import concourse.bacc as bacc
import concourse.mybir as mybir
from concourse.tile import TileContext
from concourse.bass_utils import run_bass_kernel_spmd

F32 = mybir.dt.float32
F16 = mybir.dt.float16
I16 = mybir.dt.int16
EXP = mybir.ActivationFunctionType.Exp
MULT = mybir.AluOpType.mult
ADD = mybir.AluOpType.add

T = 2048      # tokens per core (one batch element)
E = 1024      # model dim
HPC = 8       # heads per core
D = 64        # head dim
SCALE = D ** -0.5
NEC = E // 128   # 8 e-chunks
NI = 4           # i blocks of 512 (attention query cols)
NJ = 16          # j blocks of 128 (attention key rows)

# DVE-exp (Schraudolph) split: which j blocks go to the DVE instead of ScalarE.
# Spread so ScalarE never runs more than 2-3 consecutive tiles (a longer burst
# stalls the S matmuls through the 2-deep sAB PSUM rotation).
DVE_JB = frozenset((2, 4, 6, 8, 10, 12))
EXP_A = float(np.log2(np.e) / 8.0 * 1024.0)   # 184.66496...
EXP_B = 15360.0 - 44.5                        # fp16 bias 15<<10, sawtooth-centering

_CACHED_NC = None


def build_nc():
    nc = bacc.Bacc("TRN2", target_bir_lowering=False, debug=False, num_devices=8)
    x_d = nc.declare_dram_parameter("x", [T, E], F16, isOutput=False)
    wqk_d = nc.declare_dram_parameter("wqk", [E, 1024], F16, isOutput=False)
    wv_d = nc.declare_dram_parameter("wv", [E, 512], F16, isOutput=False)
    wo_d = nc.declare_dram_parameter("wo", [512, E], F16, isOutput=False)
    out_d = nc.declare_dram_parameter("out", [T, E], F32, isOutput=True)

    with TileContext(nc) as tc:
        with (
            tc.tile_pool(name="xph", bufs=1) as x_pool,
            tc.tile_pool(name="vaugp", bufs=1) as vaug_pool,
            tc.tile_pool(name="wvp", bufs=1) as wv_pool,
            tc.tile_pool(name="wop", bufs=1) as wo_pool,
            tc.tile_pool(name="otp", bufs=1) as ot_pool,
            tc.tile_pool(name="qkt", bufs=2) as qkt_pool,
            tc.tile_pool(name="wstr", bufs=4) as w_pool,
            tc.tile_pool(name="pt", bufs=6) as pt_pool,
            tc.tile_pool(name="rcp", bufs=2) as rc_pool,
            tc.tile_pool(name="rbp", bufs=2) as rb_pool,
            tc.tile_pool(name="ost2", bufs=3) as out2_pool,
            tc.tile_pool(name="qk_ps", bufs=2, space="PSUM") as qk_psum,
            tc.tile_pool(name="s_ps", bufs=2, space="PSUM") as s_psum,
            tc.tile_pool(name="oa_ps", bufs=2, space="PSUM") as oa_psum,
        ):
            # x^T as 4 per-token-chunk tiles: xTt[tcb][p, ec, t] = x[tcb*512+t,
            # ec*128+p]. Separate tiles per chunk avoid write-after-read
            # serialization between the transpose DMAs and the V/QK matmuls.
            xTt = [
                x_pool.tile([128, NEC * 512], F16, tag=f"xT{tcb}", name=f"xT{tcb}")
                for tcb in range(4)
            ]
            xTv = [t[:].rearrange("p (e t) -> p e t", t=512) for t in xTt]
            # V_aug padded to 128 cols/head: PV weight slices become 16B-
            # aligned and FWL-eligible (128 cols), so their LDWEIGHTS hide
            # behind the streaming matmuls. Cols 65..127 are zero; the extra
            # PSUM output rows 65..127 are never read.
            vaug = [
                vaug_pool.tile([128, HPC * 128], F16, tag=f"va{jb}", name=f"va{jb}")
                for jb in range(NJ)
            ]
            wv_sb = wv_pool.tile([128, NEC * 512], F16, tag="wv")
            wo_sb = [
                wo_pool.tile([128, E], F16, tag=f"wo{hc}", name=f"wo{hc}")
                for hc in range(4)
            ]
            oT = [
                ot_pool.tile([128, T], F16, tag=f"oT{hc}", name=f"oT{hc}")
                for hc in range(4)
            ]

            # ---------------- weight DMAs (gpsimd queue) --------------------
            for ec in range(NEC):
                nc.gpsimd.dma_start(
                    out=wv_sb[:, ec * 512 : (ec + 1) * 512],
                    in_=wv_d[ec * 128 : (ec + 1) * 128, :],
                )
            for hc in range(4):
                nc.gpsimd.dma_start(
                    out=wo_sb[hc][:], in_=wo_d[hc * 128 : (hc + 1) * 128, :]
                )

            # ---------------- x^T: one xbar transpose per token chunk -------
            for tcb in range(4):
                nc.sync.dma_start_transpose(
                    xTv[tcb], x_d[tcb * 512 : (tcb + 1) * 512, :]
                )

            def qk_pair_closures(pair, qp, kp):
                """Closures computing Q^T/K^T chunks for head pair `pair` into
                qp/kp (fp16). Entries: [load_w, q-mm0..3, q-evac0..3,
                k-mm0..3, k-evac0..3] -- matmul groups and their PSUM
                evacuations are separate so the evac's latency can lag."""
                wq = w_pool.tile([128, E], F16, tag="wcb", name=f"wq{pair}")
                wk = w_pool.tile([128, E], F16, tag="wcb", name=f"wk{pair}")
                cls = []

                def load_w():
                    for half, wt in ((0, wq), (1, wk)):
                        cols = slice(
                            512 * half + pair * 128, 512 * half + (pair + 1) * 128
                        )
                        nc.gpsimd.dma_start(
                            out=wt[:].rearrange("p (e c) -> p e c", c=128),
                            in_=wqk_d[:, cols].rearrange("(e p) c -> p e c", p=128),
                        )

                cls.append(load_w)
                evac = [0]
                for wcb, dst in ((wq, qp), (wk, kp)):
                    for ib in range(NI):
                        ps_ref = []

                        def grp_a(wcb=wcb, ib=ib, ps_ref=ps_ref):
                            ps = qk_psum.tile([128, 512], F32, tag="qkp")
                            ps_ref.append(ps)
                            for ec in range(4):
                                nc.tensor.matmul(
                                    ps[:],
                                    wcb[:, ec * 128 : (ec + 1) * 128],
                                    xTv[ib][:, ec, :],
                                    start=(ec == 0),
                                    stop=False,
                                )

                        def grp_b(wcb=wcb, dst=dst, ib=ib, ps_ref=ps_ref):
                            ps = ps_ref[0]
                            for ec in range(4, NEC):
                                nc.tensor.matmul(
                                    ps[:],
                                    wcb[:, ec * 128 : (ec + 1) * 128],
                                    xTv[ib][:, ec, :],
                                    start=False,
                                    stop=(ec == NEC - 1),
                                )
                            if evac[0] % 2 == 0:
                                nc.vector.tensor_copy(
                                    dst[:, ib * 512 : (ib + 1) * 512], ps[:]
                                )
                            else:
                                nc.scalar.copy(
                                    dst[:, ib * 512 : (ib + 1) * 512], ps[:]
                                )
                            evac[0] += 1

                        cls.append(grp_a)
                        cls.append(grp_b)
                return cls

            # Keep-warm filler: the PE would otherwise idle ~15us waiting for
            # the first x transpose, HAM-throttle to 1.2 GHz, and run the
            # whole head at half clock. These matmuls (garbage results into
            # the attention-phase-only oa pool, never read) execute during
            # the DMA wait and keep the clock gate at 8/8.
            for _ in range(48):
                dps = oa_psum.tile([128, 512], F32, tag="oa", name="warm")
                nc.tensor.matmul(
                    dps[:], wv_sb[:, 0:128], wv_sb[:, 0:512],
                    start=True, stop=True,
                )

            # ---------------- head: V_aug and qk pair 0 ---------------------
            def emit_v(jb):
                vview = vaug[jb][:].rearrange("p (h c) -> p h c", c=128)
                nc.vector.memset(vaug[jb][:], 0.0)
                nc.vector.memset(vview[:, :, 64:65], 1.0)
                ps = qk_psum.tile([128, 512], F32, tag="qkp")
                for ec in range(NEC):
                    nc.tensor.matmul(
                        ps[:],
                        xTv[jb // 4][:, ec, (jb % 4) * 128 : (jb % 4 + 1) * 128],
                        wv_sb[:, ec * 512 : (ec + 1) * 512],
                        start=(ec == 0),
                        stop=(ec == NEC - 1),
                    )
                nc.vector.tensor_copy(
                    vview[:, :, 0:64], ps[:].rearrange("p (h c) -> p h c", c=64)
                )

            for jb in range(NJ):
                emit_v(jb)

            qp0 = qkt_pool.tile([128, T], F16, tag="qp", name="qp0")
            kp0 = qkt_pool.tile([128, T], F16, tag="kp", name="kp0")
            cls0 = qk_pair_closures(0, qp0, kp0)
            cls0[0]()          # load_w
            cls0[1](); cls0[2]()   # q-ib0 (both halves)
            for fn in cls0[9:17]:
                fn()           # k-ib0..3
            qk0_deferred = cls0[3:9]

            # ---------------- attention + lagged norm / QKV / out-proj ------
            prj_cnt = [0]

            def prj_closures(ib):
                """out-projection closures for i-block ib (hc3 only): per
                token block, [matmul-group, evac+dma] as separate entries."""
                cls = []
                for tb in range(4 * ib, 4 * ib + 4):
                    trows = slice(tb * 128, (tb + 1) * 128)
                    for eb in range(2):
                        ecols = slice(eb * 512, (eb + 1) * 512)
                        ps_ref = []

                        def mms(trows=trows, ecols=ecols, ps_ref=ps_ref):
                            ps = qk_psum.tile([128, 512], F32, tag="qkp")
                            ps_ref.append(ps)
                            for hcc in range(4):
                                nc.tensor.matmul(
                                    ps[:],
                                    oT[hcc][:, trows],
                                    wo_sb[hcc][:, ecols],
                                    start=(hcc == 0),
                                    stop=(hcc == 3),
                                )

                        def ev(trows=trows, ecols=ecols, ps_ref=ps_ref):
                            ps = ps_ref[0]
                            ot = out2_pool.tile([128, 512], F32, tag="ost")
                            if prj_cnt[0] % 2 == 0:
                                nc.scalar.copy(ot[:], ps[:])
                            else:
                                nc.vector.tensor_copy(ot[:], ps[:])
                            deng = nc.sync if prj_cnt[0] % 2 == 0 else nc.gpsimd
                            deng.dma_start(out=out_d[trows, ecols], in_=ot[:])
                            prj_cnt[0] += 1

                        cls.append(mms)
                        cls.append(ev)
                return cls

            def norm_closures(hc, ib, oaugA, oaugB):
                """Normalization for (hc, ib): per head [stage+recip,
                broadcast, multiply] as 3 lag-queue entries."""
                icols = slice(ib * 512, (ib + 1) * 512)
                cls = []
                for oaug, rowoff in ((oaugA, 0), (oaugB, 64)):
                    st = {}

                    def recip(oaug=oaug, st=st):
                        # reciprocal_approx_fast misreads PSUM; stage via SBUF
                        dr = rc_pool.tile([1, 512], F32, tag="dr")
                        nc.vector.tensor_copy(dr[0:1, :], oaug[64:65, :])
                        rc0 = rc_pool.tile([1, 512], F32, tag="rc0")
                        nc.vector.reciprocal_approx_fast(rc0[0:1, :], dr[0:1, :])
                        st["rc0"] = rc0

                    def bcast(st=st):
                        rbs = rb_pool.tile([64, 512], F32, tag="rbs")
                        nc.gpsimd.partition_broadcast(rbs[:], st["rc0"][0:1, :])
                        st["rbs"] = rbs

                    def mul(oaug=oaug, rowoff=rowoff, st=st):
                        nc.vector.tensor_mul(
                            oT[hc][rowoff : rowoff + 64, icols],
                            oaug[0:64, :],
                            st["rbs"][:],
                        )

                    cls.extend((recip, bcast, mul))
                return cls

            qp, kp = qp0, kp0
            pending = []
            lagq = []
            for hc in range(4):
                hA, hB = 2 * hc, 2 * hc + 1
                if hc < 3:
                    qn = qkt_pool.tile([128, T], F16, tag="qp", name=f"qp{hc+1}")
                    kn = qkt_pool.tile([128, T], F16, tag="kp", name=f"kp{hc+1}")
                    pending = qk_pair_closures(hc + 1, qn, kn)
                    if hc == 0:
                        pending = qk0_deferred + pending
                else:
                    qn = kn = None
                steps = 0
                for ib in range(NI):
                    icols = slice(ib * 512, (ib + 1) * 512)
                    oaugA = oa_psum.tile([128, 512], F32, tag="oa", name="oaugA")
                    oaugB = oa_psum.tile([128, 512], F32, tag="oa", name="oaugB")
                    prev_pAB = None

                    def emit_pv(pAB, jb):
                        nc.tensor.matmul(
                            oaugA[:],
                            vaug[jb][:, hA * 128 : hA * 128 + 128],
                            pAB[:, 0:512],
                            start=(jb == 0), stop=(jb == NJ - 1),
                        )
                        nc.tensor.matmul(
                            oaugB[:],
                            vaug[jb][:, hB * 128 : hB * 128 + 128],
                            pAB[:, 512:1024],
                            start=(jb == 0), stop=(jb == NJ - 1),
                        )

                    for jb in range(NJ):
                        jcols = slice(jb * 128, (jb + 1) * 128)
                        sAB = s_psum.tile([128, 1024], F32, tag="sAB")
                        nc.tensor.matmul(
                            sAB[:, 0:512], kp[0:64, jcols], qp[0:64, icols],
                            start=True, stop=True,
                        )
                        nc.tensor.matmul(
                            sAB[:, 512:1024], kp[64:128, jcols],
                            qp[64:128, icols],
                            start=True, stop=True,
                        )
                        pAB = pt_pool.tile([128, 1024], F16, tag="pAB")
                        if jb in DVE_JB:
                            nc.vector.tensor_scalar(
                                pAB[:].bitcast(I16), sAB[:], EXP_A, EXP_B,
                                MULT, ADD,
                            )
                        else:
                            nc.scalar.activation(pAB[:], sAB[:], EXP, scale=SCALE)
                        if prev_pAB is not None:
                            emit_pv(prev_pAB, jb - 1)
                        prev_pAB = pAB
                        # lagged work: norm/out-proj first (short queue),
                        # then the next pair's QKV weave
                        steps += 1
                        if lagq:
                            lagq.pop(0)()
                        if len(lagq) > 6:
                            lagq.pop(0)()
                        if pending and steps % 3 == 0:
                            pending.pop(0)()
                        # drain oversized backlogs (hc0 carries pair0's
                        # deferred work too) before the hc boundary, where a
                        # flush would stall the PE past the HAM window
                        if len(pending) > 17 and steps % 3 == 1:
                            pending.pop(0)()
                    emit_pv(prev_pAB, NJ - 1)

                    lagq.extend(norm_closures(hc, ib, oaugA, oaugB))
                    if hc == 3:
                        lagq.extend(prj_closures(ib))
                for fn in pending:
                    fn()
                pending = []
                qp, kp = qn, kn
            for fn in lagq:
                fn()

    nc.compile()
    return nc


def get_nc():
    global _CACHED_NC
    if _CACHED_NC is None:
        _CACHED_NC = build_nc()
    return _CACHED_NC


def make_in_maps(x, w_qkv, w_out):
    in_maps = []
    for c in range(8):
        bi, hg = divmod(c, 2)
        wqk_c = np.concatenate(
            [
                w_qkv[:, hg * 512 : hg * 512 + 512],
                w_qkv[:, 1024 + hg * 512 : 1024 + hg * 512 + 512],
            ],
            axis=1,
        )
        in_maps.append(
            {
                "x": np.ascontiguousarray(x[bi]).astype(np.float16),
                "wqk": np.ascontiguousarray(wqk_c).astype(np.float16),
                "wv": np.ascontiguousarray(
                    w_qkv[:, 2048 + hg * 512 : 2048 + hg * 512 + 512]
                ).astype(np.float16),
                "wo": np.ascontiguousarray(
                    w_out[hg * 512 : hg * 512 + 512, :]
                ).astype(np.float16),
            }
        )
    return in_maps


def kernel(x, w_qkv, w_out, b_out):
    x = np.asarray(x, dtype=np.float32)
    w_qkv = np.asarray(w_qkv, dtype=np.float32)
    w_out = np.asarray(w_out, dtype=np.float32)
    b_out = np.asarray(b_out, dtype=np.float32)
    nc = get_nc()
    res = run_bass_kernel_spmd(nc, make_in_maps(x, w_qkv, w_out), list(range(8)))
    parts = [res.results[c]["out"] for c in range(8)]
    out = np.stack([parts[2 * bi] + parts[2 * bi + 1] for bi in range(4)])
    out += b_out[None, None, :]
    return out.astype(np.float32)
